# revision 36
# baseline (speedup 1.0000x reference)
"""Trainium2 Bass kernel for AlignmentModule (per-sample cross-attention).

Reference computation (per batch sample b):
    Q = W @ q + b            # (HID, HW)   1x1-conv channel matmul
    K = W @ p + b            # (HID, HW)
    S = Q^T K                # (HW, HW)
    A = softmax(S, axis=-1)
    aligned = V @ A^T        # (C, HW), V = p
    out = concat([q, aligned], channel axis)

Strategy: data-parallel over batch across 8 NeuronCores (2 samples/core).
Projections stream q/p in fp16 (bias applied on ScalarE, which is idle
during projections). Softmax runs on transposed scores S^T (k on
partitions): exp with a constant -40 shift into bf16, pairwise DVE
fold-sums pipelined into the kb loop + a ones-matmul on TensorE for the
column sums, then weights are normalized to (0,1] and cast to fp8e4m3.

ALL 8 key blocks run the AV einsum as fp8 DoubleRow matmuls (4 N=512
instructions per 128-channel block instead of 5 in the older
6-fp8/2-bf16 mix). The fp8 precision cost is bought back by a
quantized-sum renormalization: a ones-DoubleRow matmul sums the
*quantized* weights per query (into the acc-pool A3 bank - putting it
in the AV psum ring deadlocks the Tile scheduler), and the PSUM->SBUF
evacuation becomes a DVE multiply by the reciprocal of that sum. This
cancels the common-mode (per-query scaling) part of the fp8 weight
error, which dominates for peaked softmax: CPU-sim (matches HW to
~5e-6) gives rel_err 1.825e-2 vs 1.842e-2 for the old mix, while
saving 64 matmuls (~14us).

Engine budget per scores h-pass: ACT exp chain 5.5us is the pacer; PE
own work is 3.9us, so exactly one AV filler unit is interleaved per
h-pass (more would flood the DVE, whose normalize+folds+evacs already
run ~7us per pass). The s8 jobs sit at the late kb slots of the NEXT
h-pass so their DVE normalize dependency is ready just in time.
Out-DMA issues ride the sync queue (input issues are long done);
GpSimd is avoided entirely (its tensor ops measure ~10x slower than
DVE). Input DMAs ride the sync queue in consumption order
(q0,p0,vt8(0),q1,p1,vt8(1)); wt/bias on scalar.
"""

import sys

if "/opt/trn_rl_repo" not in sys.path:
    sys.path.insert(0, "/opt/trn_rl_repo")

import ml_dtypes
import numpy as np

import concourse.bass as bass
import concourse.mybir as mybir
import concourse.tile as tile
from concourse import bacc
from concourse.bass_utils import run_bass_kernel_spmd

import os
NO_WARMUP = bool(int(os.environ.get("KERNEL_NO_WARMUP", "0")))
NWARM = int(os.environ.get("KERNEL_WARMUP", "6"))

B, C, HID, H, W_ = 16, 2048, 256, 32, 32
HW = H * W_            # 1024
NCORES = 8
BS = B // NCORES       # samples per core
P = 128
CT = C // P            # 16 channel tiles
OT = HID // P           # 2 hid blocks
KT = HW // P            # 8 key blocks
NH = 2                 # free-dim halves of HW
NF = HW // NH          # 512 (PSUM bank / fp32 moving-operand max)
NU = KT // 2           # DoubleRow pair count (all 8 key blocks fp8)

F32 = mybir.dt.float32
F16 = mybir.dt.float16
BF16 = mybir.dt.bfloat16
F8 = mybir.dt.float8e4
DR = mybir.MatmulPerfMode.DoubleRow
EXP_SHIFT = -40.0

_NC_CACHE = None
LAST_RESULTS = None


def _ensure_ntff_hook():
    """Register the axon NTFF profile hook if the image's antenv lacks it."""
    import types

    try:
        from antenv.axon_hooks import get_axon_ntff_profile_hook  # noqa: F401
        return
    except ImportError:
        pass
    try:
        from trn_agent_boot.trn_boot import _ntff_profile_via_ctypes
    except ImportError:
        return
    hook = _ntff_profile_via_ctypes("/opt/axon/libaxon_pjrt.so")
    mod = types.ModuleType("antenv.axon_hooks")
    mod._hook = hook
    mod.get_axon_ntff_profile_hook = lambda: mod._hook
    mod.set_axon_ntff_profile_hook = lambda h: setattr(mod, "_hook", h)
    sys.modules["antenv.axon_hooks"] = mod
    import antenv

    antenv.axon_hooks = mod


def _build_nc():
    nc = bacc.Bacc(None, target_bir_lowering=False)

    q_d = nc.declare_dram_parameter("q", [BS, C, HW], F16, isOutput=False)
    p_d = nc.declare_dram_parameter("p", [BS, C, HW], F16, isOutput=False)
    pt8_d = nc.declare_dram_parameter("pt8", [BS, HW, C], F8, isOutput=False)
    wt_d = nc.declare_dram_parameter("wt", [C, HID], F16, isOutput=False)
    b_d = nc.declare_dram_parameter("b2", [P, OT], F32, isOutput=False)
    out_d = nc.declare_dram_parameter("out", [BS, C, HW], BF16, isOutput=True)

    Ident = mybir.ActivationFunctionType.Identity
    Exp = mybir.ActivationFunctionType.Exp

    with tile.TileContext(nc) as tc:
        with (
            tc.tile_pool(name="const", bufs=1) as const_pool,
            tc.tile_pool(name="xstream", bufs=12) as x_pool,
            tc.tile_pool(name="vt", bufs=2) as vt_pool,
            tc.tile_pool(name="qf", bufs=1) as qf_pool,
            tc.tile_pool(name="kf", bufs=1) as kf_pool,
            tc.tile_pool(name="e", bufs=1) as e_pool,
            tc.tile_pool(name="w", bufs=2) as w_pool,
            tc.tile_pool(name="rb", bufs=2) as rb_pool,
            tc.tile_pool(name="fold", bufs=2) as fold_pool,
            tc.tile_pool(name="ostage", bufs=1) as o_pool,
            tc.tile_pool(name="acc_ps", bufs=1, space="PSUM") as acc_psum,
            tc.tile_pool(name="av_ps", bufs=4, space="PSUM") as av_psum,
        ):
            # PE warm-up: cold-rate dummy matmuls so the HAM clock gate
            # opens while the first input DMAs are still in flight.
            wu_src = const_pool.tile([P, NF], BF16)
            nc.any.memset(wu_src[:], 0.0)
            wu_sink = const_pool.tile([P, 1], F32)

            def emit_fill(n):
                fil = av_psum.tile([P, NF], F32, name="avp")
                for i in range(n):
                    nc.tensor.matmul(
                        fil[:],
                        wu_src[:, :P],
                        wu_src[:],
                        start=(i == 0),
                        stop=(i == n - 1),
                    )
                nc.vector.tensor_copy(wu_sink[:], fil[:, :1])

            if not NO_WARMUP and NWARM:
                emit_fill(NWARM)

            wt_r = wt_d.rearrange("(a p) o -> p a o", p=P)
            wt_s = const_pool.tile([P, CT, HID], F16)
            for lo, hi in ((0, 2), (2, 4)):
                nc.scalar.dma_start(wt_s[:, lo:hi, :], wt_r[:, lo:hi, :])
            # later wt chunks are paced between the q-stream issues on the
            # sync queue (emitted inside emit_proj(0)) so they don't starve
            # the first xt transfers
            wt_pend = [(4, 8), (8, 12), (12, 16)]

            def emit_wt_chunk():
                if wt_pend:
                    lo, hi = wt_pend.pop(0)
                    nc.sync.dma_start(wt_s[:, lo:hi, :], wt_r[:, lo:hi, :])

            b_s = const_pool.tile([P, OT], F32)
            nc.scalar.dma_start(b_s[:], b_d[:])
            ones_s = const_pool.tile([P, P], BF16)
            nc.any.memset(ones_s[:], 1.0)
            ones8 = const_pool.tile([P, 2, P], F8)
            nc.any.memset(ones8[:], 1.0)
            shift_s = const_pool.tile([P, 1], F32)
            nc.any.memset(shift_s[:], EXP_SHIFT)

            vt_tiles = {}

            def _emit_vt8(s):
                vt8 = vt_pool.tile([P, NU, 2, C], F8, name="vt8")
                pt8_r = pt8_d[s].rearrange("(u i p) c -> p u i c", u=NU, i=2, p=P)
                for u in range(NU):
                    nc.sync.dma_start(vt8[:, u:u + 1, :, :], pt8_r[:, u:u + 1, :, :])
                vt_tiles[s] = vt8

            samples = [dict() for _ in range(BS)]

            def emit_proj(s, fillers=None):
                fillers = fillers or []
                st = samples[s]
                st["qf"] = qf_pool.tile([P, OT, HW], F16, name="qf")
                st["kf"] = kf_pool.tile([P, OT, HW], F16, name="kf")
                for src, dst in ((q_d, st["qf"]), (p_d, st["kf"])):
                    src_r = src[s].rearrange("(a p) f -> p a f", p=P)
                    pj = [
                        [
                            acc_psum.tile([P, NF], F32, name=f"A{2 * j + h}")
                            for h in range(NH)
                        ]
                        for j in range(OT)
                    ]
                    for u in range(CT // 2):
                        xt = x_pool.tile([P, 2, HW], F16, name="xp")
                        nc.sync.dma_start(xt[:], src_r[:, 2 * u:2 * u + 2, :])
                        if s == 0 and src is q_d and u in (1, 2, 3):
                            emit_wt_chunk()
                        for du in range(2):
                            t = 2 * u + du
                            for j in range(OT):
                                for h in range(NH):
                                    nc.tensor.matmul(
                                        pj[j][h][:],
                                        wt_s[:, t, j * P:(j + 1) * P],
                                        xt[:, du, h * NF:(h + 1) * NF],
                                        start=(t == 0),
                                        stop=(t == CT - 1),
                                    )
                        # DMA-ramp insurance at kernel start only
                        if s == 0 and src is q_d and u in (0, 1, 2, 3, 4):
                            emit_fill(2)
                        # AV units of the previous sample ride the DVE-idle
                        # projection window (their evacuations are free here)
                        if fillers and u % 2 == 1:
                            fillers.pop(0)()
                    # bias on ScalarE (idle during projections)
                    for h in range(NH):
                        for j in range(OT):
                            nc.scalar.activation(
                                dst[:, j, h * NF:(h + 1) * NF],
                                pj[j][h][:],
                                Ident,
                                bias=b_s[:, j:j + 1],
                                scale=1.0,
                            )
                # V^T for the AV phase, ordered after this sample's q/p
                # streams on the same (sync) DMA queue.
                _emit_vt8(s)

            def make_s8job(s, h):
                """Sum the quantized weights (4 ones-DoubleRow matmuls into
                the A3 bank) and write the per-query renorm reciprocal.
                Scheduled late enough that normalize(s,h) has finished."""
                def job():
                    st = samples[s]
                    w8 = st["w8"]
                    s8p = acc_psum.tile([P, NF], F32, name="A3")
                    for u in range(NU):
                        nc.tensor.matmul(
                            s8p[:],
                            ones8[:],
                            w8[:, u, :, h * NF:(h + 1) * NF],
                            start=(u == 0),
                            stop=(u == NU - 1),
                            perf_mode=DR,
                        )
                    nc.vector.reciprocal_approx_fast(st["rcorr"][:, h, :], s8p[:])
                return job

            def emit_scores(s, fillers=None):
                """fillers: list of callables (or None) popped at the kb-odd
                slots and one tail slot of each h-pass (5 slots per h)."""
                fillers = fillers or []

                def pop():
                    if fillers:
                        f = fillers.pop(0)
                        if f is not None:
                            f()

                st = samples[s]
                qf, kf = st["qf"], st["kf"]
                e = e_pool.tile([P, NU, 2, HW], BF16, name="e")
                w8 = w_pool.tile([P, NU, 2, HW], F8, name="w8")
                rb = rb_pool.tile([P, NH, 1, 1, NF], F32, name="rb")
                rcorr = rb_pool.tile([P, NH, NF], F32, name="rc")
                st["e"], st["w8"], st["rcorr"] = e, w8, rcorr

                def norm_half(h, ulo, uhi, recip=False):
                    hs = slice(h * NF, (h + 1) * NF)
                    if recip:
                        nc.vector.reciprocal_approx_fast(
                            rb[:, h, 0, 0, :], st["smp"][h][:]
                        )
                    nc.vector.tensor_mul(
                        w8[:, ulo:uhi, :, hs],
                        e[:, ulo:uhi, :, hs],
                        rb[:, h].broadcast_to([P, uhi - ulo, 2, NF]),
                    )

                st["smp"] = {}
                norm_jobs = []
                for h in range(NH):
                    hs = slice(h * NF, (h + 1) * NF)
                    smp = acc_psum.tile([P, NF], F32, name="A3")
                    st["smp"][h] = smp
                    fA = fold_pool.tile([P, NU, NF], BF16, name="fA")
                    fB = fold_pool.tile([P, 2, NF], BF16, name="fB")
                    fC = fold_pool.tile([P, NF], BF16, name="fC")
                    for kb in range(KT):
                        stp = acc_psum.tile([P, NF], F32, name=f"A{kb % 3}")
                        for j in range(OT):
                            nc.tensor.matmul(
                                stp[:],
                                kf[:, j, kb * P:(kb + 1) * P],
                                qf[:, j, hs],
                                start=(j == 0),
                                stop=(j == OT - 1),
                            )
                        nc.scalar.activation(
                            e[:, kb // 2, kb % 2, hs],
                            stp[:],
                            Exp,
                            bias=shift_s[:],
                            scale=1.0,
                        )
                        if kb % 2 == 1:
                            u = kb // 2
                            # pairwise fold of the two blocks just exp'd
                            nc.vector.tensor_add(
                                fA[:, u, :], e[:, u, 0, hs], e[:, u, 1, hs]
                            )
                            pop()
                    pop()
                    nc.vector.tensor_add(fB[:], fA[:, 0:2, :], fA[:, 2:4, :])
                    nc.vector.tensor_add(fC[:], fB[:, 0, :], fB[:, 1, :])
                    nc.tensor.matmul(
                        smp[:], ones_s[:], fC[:], start=True, stop=True
                    )
                    if h < NH - 1:
                        norm_half(h, 0, NU, recip=True)
                    else:
                        # defer the last normalize in halves: emitted by the
                        # caller between AV units so the 2x2.3us DVE ops
                        # don't block the AV evacuations queued behind them
                        norm_jobs = [
                            lambda h=h: norm_half(h, 0, 2, recip=True),
                            lambda h=h: norm_half(h, 2, NU),
                        ]
                while fillers:
                    pop()
                return norm_jobs

            def emit_av_unit(s, h, cp, alt=False):
                st = samples[s]
                vt8 = vt_tiles[s]
                w8, rcorr = st["w8"], st["rcorr"]
                if "ots" not in st:
                    st["ots"] = [
                        o_pool.tile([P, 2, HW], BF16, name=f"ot{c}")
                        for c in range(CT // 2)
                    ]
                    st["out_r"] = out_d[s].rearrange("(a p) f -> p a f", p=P)
                ot = st["ots"][cp]
                out_r = st["out_r"]
                for dc in range(2):
                    cb = 2 * cp + dc
                    # alt units borrow the (idle post-scores) acc banks,
                    # deepening the psum lookahead to ~3 units so DVE
                    # normalize bursts don't stall the PE on the ring
                    if alt:
                        avp = acc_psum.tile([P, NF], F32, name=f"A{dc}")
                    else:
                        avp = av_psum.tile([P, NF], F32, name="avp")
                    for u in range(NU):
                        nc.tensor.matmul(
                            avp[:],
                            vt8[:, u, :, cb * P:(cb + 1) * P],
                            w8[:, u, :, h * NF:(h + 1) * NF],
                            start=(u == 0),
                            stop=(u == NU - 1),
                            perf_mode=DR,
                        )
                    # evacuate with the fp8-sum renormalization folded in
                    nc.vector.tensor_mul(
                        ot[:, dc, h * NF:(h + 1) * NF],
                        avp[:],
                        rcorr[:, h, :],
                    )
                    if s == BS - 1 and h == NH - 1 and cp == CT // 2 - 1:
                        # final unit: issue each half right after its
                        # evacuation so the tail transfer is minimal
                        nc.sync.dma_start(
                            out_r[
                                :,
                                2 * cp + dc:2 * cp + dc + 1,
                                h * NF:(h + 1) * NF,
                            ],
                            ot[:, dc:dc + 1, h * NF:(h + 1) * NF],
                        )
                # store each h-half as soon as it is complete: halves the
                # tail transfer after the last matmul and spreads the out
                # traffic into the (input-quiet) AV windows
                hsl = slice(h * NF, (h + 1) * NF)
                if not (s == BS - 1 and h == NH - 1 and cp == CT // 2 - 1):
                    nc.sync.dma_start(
                        out_r[:, 2 * cp:2 * cp + 2, hsl], ot[:, :, hsl]
                    )

            # ---- schedule ----
            # filler slot order per emit_scores:
            #   h0: kb1, kb3, kb5, kb7, tail | h1: kb1, kb3, kb5, kb7, tail
            emit_proj(0)
            # scores0: nothing to interleave; only the s8 job for h0,
            # placed at h1-kb7 (after the DVE normalize of h0 drains).
            nj0 = emit_scores(0, [None, None, None, None, None,
                                  None, None, None, make_s8job(0, 0), None])
            emit_av_unit(0, 0, 0)
            nj0[0]()
            emit_av_unit(0, 0, 1)
            nj0[1]()
            emit_av_unit(0, 0, 2)
            make_s8job(0, 1)()
            for cp in range(3, CT // 2):
                emit_av_unit(0, 0, cp)
            av01 = [
                lambda alt=False, cp=cp: emit_av_unit(0, 1, cp, alt=alt)
                for cp in range(CT // 2)
            ]
            # av01[3:] ride proj1's DVE-idle window (avp ring only - the
            # acc banks belong to the projection accumulators there)
            emit_proj(1, [av01[3], av01[4], av01[5], av01[6], av01[7]])
            # one AV filler per h-pass (ACT exp chain leaves ~1.6us of PE
            # idle per pass; more fillers would flood the DVE with evacs
            # ahead of the normalize it must finish first)
            nj1 = emit_scores(1, [None, None, av01[0], None, None,
                                  None, av01[1], None, make_s8job(1, 0), av01[2]])
            emit_av_unit(1, 0, 0)
            nj1[0]()
            emit_av_unit(1, 0, 1, alt=True)
            nj1[1]()
            emit_av_unit(1, 0, 2)
            make_s8job(1, 1)()
            for cp in range(3, CT // 2):
                emit_av_unit(1, 0, cp, alt=(cp % 2 == 1))
            for cp in range(CT // 2):
                emit_av_unit(1, 1, cp, alt=(cp % 2 == 1))

    nc.compile()
    return nc


def _get_nc():
    global _NC_CACHE
    if _NC_CACHE is None:
        _NC_CACHE = _build_nc()
    return _NC_CACHE


def kernel(query_features, prompt_features, W, b, _profile=False):
    global LAST_RESULTS
    qv = np.asarray(query_features, dtype=np.float32).reshape(B, C, HW)
    pv = np.asarray(prompt_features, dtype=np.float32).reshape(B, C, HW)
    q16 = np.ascontiguousarray(qv).astype(np.float16)
    p16 = np.ascontiguousarray(pv).astype(np.float16)
    pt8 = np.ascontiguousarray(pv.transpose(0, 2, 1)).astype(ml_dtypes.float8_e4m3)
    wt = np.ascontiguousarray(np.asarray(W, dtype=np.float32).T).astype(np.float16)
    b2 = np.ascontiguousarray(np.asarray(b, dtype=np.float32).reshape(OT, P).T)

    if _profile:
        _ensure_ntff_hook()
    nc = _get_nc()
    in_maps = []
    for i in range(NCORES):
        sl = slice(i * BS, (i + 1) * BS)
        in_maps.append(
            {"q": q16[sl], "p": p16[sl], "pt8": pt8[sl], "wt": wt, "b2": b2}
        )
    res = run_bass_kernel_spmd(
        nc, in_maps, core_ids=list(range(NCORES)), trace=_profile
    )
    LAST_RESULTS = res
    aligned = np.concatenate(
        [np.asarray(r["out"], dtype=np.float32) for r in res.results], axis=0
    )
    aligned = aligned.reshape(B, C, H, W_)
    full = np.concatenate(
        [np.asarray(query_features, dtype=np.float32).reshape(B, C, H, W_), aligned],
        axis=1,
    )
    return full
